# revision 64
# baseline (speedup 1.0000x reference)
"""Trainium2 Bass kernel for multi-head attention decode step with KV cache.

Problem shapes (hardcoded): x[16,32,4096], wq/wk/wv/wo[4096,4096],
k_cache/v_cache[16,2048,32,128], start_pos=1024 (must be multiple of 128).

Sharding: tensor-parallel over the 32 heads -> 4 heads per core on 8 cores.
wq/wk/wv column-sharded, wo row-sharded; per-core partial outputs (bf16)
summed on host.

Numerics (identical to the validated baseline, rel err ~1.4e-2):
  - x and all of wq/wk/wv/wo split hi/lo in fp8-e4m3; projections are fp8
    DoubleRow matmuls in three full-K sweeps (w_lo x x_hi, w_hi x x_hi,
    w_hi x x_lo).
  - Weights pre-scaled x256 on host; the 1/256 folds into rope tables /
    eviction scales. K/V caches e3m4 x2. probs bf16, denominators via
    ones-matmuls sharing the A.V psum tile. Attention out hi/lo e4m3 x32,
    3-pass DoubleRow wo, bf16 partials summed fp32 on host.

Scheduling: each DMA's transfer time is charged to the issuing engine's
queue and engines greedily run ready work in emission-priority order, so
DMAs are spread across queues to keep the tensor engine the only wall:
SP carries the weight stream (quarter tiles, one DoubleRow pair each)
and output DMAs; Act carries rope tables + wo weights + the exp stream;
Pool (gpsimd) carries x (quarter tiles) + the whole K/V cache stream;
DVE carries rope, V-psum evictions, per-batch-group normalize+quantize,
and A.V psum staging. New-V rows live in 4 merged [128,512] tiles (row
offset 32*(b%4), re-based fixup copies for b%4==3 since matmul operands
must start at partition 0/32/64). The attention (b,h) loop is
software-pipelined two pairs deep; the wo projection runs in token-span
passes (one per 2 batches, per-batch for b0/b1) emitted right after each
span's normalize+quantize so the tensor engine always has fill work
while exp paces the attention stream, with the final output DMAs split
per-block across SP/Act/Pool to shorten the end-of-kernel tail.
"""

import numpy as np

B, T, D = 16, 32, 4096
HL, HD = 4, 128          # heads per core, head dim
NTOK = B * T             # 512
NC = 8
SW = 256.0               # host weight scale (folded back on-chip)
SA = 32.0                # attn quantization scale (folded into bcast+evict)
SKV = 2.0                # e3m4 cache scale

_STATE = {}


def _build(n_cached):
    import concourse.tile as tile
    from concourse import bacc, mybir
    from contextlib import ExitStack

    f32 = mybir.dt.float32
    bf16 = mybir.dt.bfloat16
    e4 = mybir.dt.float8e4
    e3 = mybir.dt.float8e3
    DR = mybir.MatmulPerfMode.DoubleRow
    Exp = mybir.ActivationFunctionType.Exp

    SCF = n_cached // 128          # full cached s-chunks (8)
    COLS = SCF * 32 + 32           # scoresT free width (288)
    CW = HL * HD                   # per-core projection width (512)

    nc = bacc.Bacc("TRN2", target_bir_lowering=False, debug=False,
                   num_devices=NC)

    xh = nc.dram_tensor("xh", [D, NTOK], e4, kind="ExternalInput").ap()
    xl = nc.dram_tensor("xl", [D, NTOK], e4, kind="ExternalInput").ap()
    wqh = nc.dram_tensor("wqh", [D, CW], e4, kind="ExternalInput").ap()
    wql = nc.dram_tensor("wql", [D, CW], e4, kind="ExternalInput").ap()
    wkh = nc.dram_tensor("wkh", [D, CW], e4, kind="ExternalInput").ap()
    wkl = nc.dram_tensor("wkl", [D, CW], e4, kind="ExternalInput").ap()
    wvh = nc.dram_tensor("wvh", [D, CW], e4, kind="ExternalInput").ap()
    wvl = nc.dram_tensor("wvl", [D, CW], e4, kind="ExternalInput").ap()
    woh = nc.dram_tensor("woh", [CW, D], e4, kind="ExternalInput").ap()
    wol = nc.dram_tensor("wol", [CW, D], e4, kind="ExternalInput").ap()
    ktc = nc.dram_tensor("ktc", [B, HL, HD, n_cached], e3,
                         kind="ExternalInput").ap()
    vcc = nc.dram_tensor("vcc", [B, HL, 128, SCF * HD], e3,
                         kind="ExternalInput").ap()
    cosd = nc.dram_tensor("cosd", [128, NTOK], f32, kind="ExternalInput").ap()
    sind = nc.dram_tensor("sind", [128, NTOK], f32, kind="ExternalInput").ap()
    coskd = nc.dram_tensor("coskd", [128, NTOK], f32,
                           kind="ExternalInput").ap()
    sinkd = nc.dram_tensor("sinkd", [128, NTOK], f32,
                           kind="ExternalInput").ap()
    outp = nc.dram_tensor("outp", [D, NTOK], bf16, kind="ExternalOutput").ap()

    SCALE = float(1.0 / np.sqrt(np.float32(HD)))

    with tile.TileContext(nc) as tc:
        with ExitStack() as outer:
            qk_pool = outer.enter_context(tc.tile_pool(name="qk", bufs=1))
            vb_pool = outer.enter_context(tc.tile_pool(name="vnb", bufs=1))
            wo_pool = outer.enter_context(tc.tile_pool(name="wo", bufs=1))
            cs_pool = outer.enter_context(tc.tile_pool(name="cs", bufs=1))
            one_pool = outer.enter_context(tc.tile_pool(name="ones", bufs=1))
            ath_pool = outer.enter_context(tc.tile_pool(name="ath", bufs=1))
            ahl_pool = outer.enter_context(tc.tile_pool(name="ahl", bufs=1))
            kc_pool = outer.enter_context(tc.tile_pool(name="kc", bufs=8))
            vc_pool = outer.enter_context(tc.tile_pool(name="vc", bufs=6))
            out_pool = outer.enter_context(tc.tile_pool(name="outsb", bufs=1))

            ones_mat = one_pool.tile([128, 128], bf16, tag="oc", name="oc")
            nc.vector.memset(ones_mat[:], 1.0)

            QT = [qk_pool.tile([128, NTOK], bf16, tag=f"q{m}", name=f"QT{m}")
                  for m in range(HL)]
            KT = [qk_pool.tile([128, NTOK], bf16, tag=f"k{m}", name=f"KT{m}")
                  for m in range(HL)]
            # new-V rows (x SKV): one merged tile per projection group m,
            # rows 32*(b%4) hold batch b = 4*m + b%4. Matmul operands may
            # only start at partition 0/32/64, so the b%4==3 rows (base 96)
            # are re-based into two fixup tiles: VNB3A rows 32*m for
            # m=0,1,2 and VNB3B rows 0 for m=3.
            VNB4 = [vb_pool.tile([128, CW], bf16, tag=f"vb{m}",
                                 name=f"VNB{m}") for m in range(HL)]
            VNB3A = vb_pool.tile([128, CW], bf16, tag="vb3a", name="VNB3A")
            VNB3B = vb_pool.tile([128, CW], bf16, tag="vb3b", name="VNB3B")
            OFF = [(32 * (b % 4)) if b % 4 < 3 else
                   (32 * (b // 4) if b // 4 < 3 else 0) for b in range(B)]

            def vnb_new(b):
                # (tile, row offset) holding batch b's new-V rows
                m, r = b // 4, b % 4
                if r < 3:
                    return VNB4[m], 32 * r
                return (VNB3A, 32 * m) if m < 3 else (VNB3B, 0)
            # per-b raw A.V [128, (h,32)] + denominators (row 0, cols
            # 128:256): [128, (b, 256)] f32
            ATH3 = ath_pool.tile([128, B * 256], f32, tag="at", name="at")
            # attn quantized hi/lo, head-pair interleaved for DoubleRow wo
            AH2 = [ahl_pool.tile([128, 2 * NTOK], e4, tag=f"ah{g}",
                                 name=f"ah{g}") for g in range(2)]
            AL2 = [ahl_pool.tile([128, 2 * NTOK], e4, tag=f"al{g}",
                                 name=f"al{g}") for g in range(2)]
            # resident wo (hi/lo): [128, (h, D)]
            WOH = wo_pool.tile([128, HL * D], e4, tag="woh", name="woh")
            WOL = wo_pool.tile([128, HL * D], e4, tag="wol", name="wol")

            # K/V cache tiles (outer scope; streamed)
            KTS, VTS = [], []

            def k_dma(b):
                kt = kc_pool.tile([128, HL * n_cached], e3, tag="kt",
                                  name="kt")
                nc.gpsimd.dma_start(
                    kt[:].rearrange("p (h s) -> p h s", h=HL),
                    ktc[b].rearrange("h p s -> p h s"))
                KTS.append(kt)

            def v_dma(b):
                vt = vc_pool.tile([128, HL * SCF * HD], e3, tag="vt",
                                  name="vt")
                nc.gpsimd.dma_start(
                    vt[:].rearrange("p (h s) -> p h s", h=HL),
                    vcc[b].rearrange("h p s -> p h s"))
                VTS.append(vt)

            # ---------------- Phase A: projections + rope ----------------
            with ExitStack() as pa:
              if True:
                  x_pool = pa.enter_context(tc.tile_pool(name="xt", bufs=1))
                  w_pool = pa.enter_context(tc.tile_pool(name="w", bufs=14))
                  pp = pa.enter_context(
                      tc.tile_pool(name="pp", bufs=2, space="PSUM"))
                  rope_pool = pa.enter_context(tc.tile_pool(name="rope",
                                                            bufs=2))

                  # x quarter-tiles: [128, 2, NTOK] (one DoubleRow pair),
                  # all on the Pool queue: xh first, then xl
                  def load_x(dram, g, q, nm):
                      t = x_pool.tile([128, 2 * NTOK], e4,
                                      tag=f"{nm}{g}q{q}",
                                      name=f"{nm}{g}q{q}")
                      base = g * 1024 + q * 256
                      src = dram[base:base + 256, :].rearrange(
                          "(ks p) n -> p ks n", p=128)
                      nc.gpsimd.dma_start(
                          t[:].rearrange("p (a b) -> p a b", a=2), src)
                      return t

                  XH = [[load_x(xh, g, q, "xh") for q in range(4)]
                        for g in range(4)]
                  XL = [[load_x(xl, g, q, "xl") for q in range(4)]
                        for g in range(4)]
                  cosk = cs_pool.tile([128, NTOK], f32, tag="cosk",
                                      name="cosk")
                  sink = cs_pool.tile([128, NTOK], f32, tag="sink",
                                      name="sink")
                  nc.gpsimd.dma_start(cosk[:], coskd[:])
                  nc.gpsimd.dma_start(sink[:], sinkd[:])
                  for b in range(6):
                      k_dma(b)
                  for b in range(4):
                      v_dma(b)
                  for b in range(6, B):
                      k_dma(b)

                  # Act queue: cos/sin + resident wo weights
                  cos_sb = cs_pool.tile([128, NTOK], f32, tag="cos",
                                        name="cos")
                  sin_sb = cs_pool.tile([128, NTOK], f32, tag="sin",
                                        name="sin")
                  nc.scalar.dma_start(cos_sb[:], cosd[:])
                  nc.scalar.dma_start(sin_sb[:], sind[:])
                  for h in range(HL):
                      nc.scalar.dma_start(WOH[:, h * D:(h + 1) * D],
                                          woh[h * 128:(h + 1) * 128, :])
                      nc.scalar.dma_start(WOL[:, h * D:(h + 1) * D],
                                          wol[h * 128:(h + 1) * 128, :])

                  # w quarter-tiles on SP: [128, 2, CW] (one DR pair)
                  def load_w(wap, g, q):
                      t = w_pool.tile([128, 2 * CW], e4, tag="w",
                                      name="wtile")
                      base = g * 1024 + q * 256
                      nc.sync.dma_start(
                          t[:].rearrange("p (a b) -> p a b", a=2),
                          wap[base:base + 256, :].rearrange(
                              "(ks p) n -> p ks n", p=128))
                      return t

                  def rope_evict(ps, dst, ct, st):
                      # swapped-half muls via partition offsets; cos/sin carry
                      # the 1/SW fold (sin table first half negated)
                      tco = rope_pool.tile([128, NTOK], f32, tag="tco",
                                           name="tco")
                      nc.vector.tensor_mul(tco[:], ps[:], ct[:])
                      tsi = rope_pool.tile([128, NTOK], f32, tag="tsi",
                                           name="tsi")
                      nc.vector.tensor_mul(tsi[0:64, :], ps[64:128, :],
                                           st[0:64, :])
                      nc.vector.tensor_mul(tsi[64:128, :], ps[0:64, :],
                                           st[64:128, :])
                      nc.vector.tensor_add(dst[:], tco[:], tsi[:])

                  def r3(t):
                      return t[:].rearrange("p (a b) -> p a b", a=2)

                  def proj(wlo_d, whi_d, swap_lhs):
                      """Streamed hi/lo projection into 4 [128, NTOK] psums.
                      Sweep A: w_lo x x_hi; sweep B: w_hi x {x_hi, x_lo}.
                      swap_lhs: V orientation (lhsT = x, rhs = w)."""
                      ps = [pp.tile([128, NTOK], f32, tag=f"pp{m}",
                                    name=f"pp{m}") for m in range(HL)]

                      def mm(wt, xt, m, first, last):
                          if swap_lhs:
                              wsl = r3(wt)[:, :, :]
                              xsl = r3(xt)[:, :, m * 128:(m + 1) * 128]
                              nc.tensor.matmul(ps[m][:], xsl, wsl,
                                               start=first, stop=last,
                                               perf_mode=DR)
                          else:
                              wsl = r3(wt)[:, :, m * 128:(m + 1) * 128]
                              xsl = r3(xt)[:, :, :]
                              nc.tensor.matmul(ps[m][:], wsl, xsl,
                                               start=first, stop=last,
                                               perf_mode=DR)

                      for g in range(4):
                          for j in range(4):
                              wt = load_w(wlo_d, g, j)
                              for m in range(HL):
                                  mm(wt, XH[g][j], m,
                                     (g == 0 and j == 0), False)
                      for g in range(4):
                          for j in range(4):
                              wt = load_w(whi_d, g, j)
                              for m in range(HL):
                                  last = (g == 3 and j == 3)
                                  for xi, xsrc in enumerate((XH, XL)):
                                      mm(wt, xsrc[g][j], m, False,
                                         (last and xi == 1))
                      return ps

                  # --- Q ---
                  psq = proj(wql, wqh, False)
                  for m in range(HL):
                      rope_evict(psq[m], QT[m], cos_sb, sin_sb)
                  # --- K ---
                  psk = proj(wkl, wkh, False)
                  for m in range(HL):
                      rope_evict(psk[m], KT[m], cosk, sink)
                  # --- V ([tok, cols] orientation) ---
                  psv = proj(wvl, wvh, True)
                  # one wide eviction per m, alternating DVE/Act so the V
                  # psum banks free ASAP for phase B reuse (Act is idle
                  # until the first scores finish); the b%4==3 re-base
                  # fixups are cheap SBUF->SBUF copies in the b loop
                  for m in range(HL):
                      if m % 2 == 0:
                          nc.vector.tensor_scalar_mul(
                              VNB4[m][:], psv[m][:], SKV / SW)
                      else:
                          nc.scalar.mul(VNB4[m][:], psv[m][:], SKV / SW)

            # ------------- Phase B: attention + psum-accumulated wo -------
            with ExitStack() as pb:
              if True:
                  pr_pool = pb.enter_context(tc.tile_pool(name="probs",
                                                          bufs=5))
                  rb_pool = pb.enter_context(tc.tile_pool(name="rb", bufs=2))
                  pt_pool = pb.enter_context(tc.tile_pool(name="pt", bufs=2))
                  sc_ps = pb.enter_context(
                      tc.tile_pool(name="scps", bufs=3, space="PSUM"))
                  av_ps = pb.enter_context(
                      tc.tile_pool(name="avps", bufs=3, space="PSUM"))
                  po_ps = pb.enter_context(
                      tc.tile_pool(name="pops", bufs=2, space="PSUM"))

                  for b in range(4, B):
                      v_dma(b)

                  aps_of = {}   # b -> av psum tile

                  def emit_scores_exp(b, h):
                      kt = KTS[b]
                      o = OFF[b]
                      c0 = b * 32 - o
                      wlen = min(128, NTOK - c0)
                      qs = QT[h][:, b * 32:(b + 1) * 32]
                      sp = sc_ps.tile([128, COLS], f32, tag="sp", name="sp")
                      for sc in range(SCF):
                          nc.tensor.matmul(
                              sp[:, sc * 32:(sc + 1) * 32],
                              kt[:, h * n_cached + sc * 128:
                                 h * n_cached + (sc + 1) * 128],
                              qs, start=True, stop=True)
                      # 128-token K window fills all partitions so one exp
                      # covers everything; valid block at rows o:o+32
                      nc.tensor.matmul(
                          sp[0:wlen, SCF * 32:COLS],
                          KT[h][:, c0:c0 + wlen], qs,
                          start=True, stop=True)

                      pr = pr_pool.tile([128, COLS], bf16, tag="pr",
                                        name="pr")
                      if wlen == 128:
                          nc.scalar.activation(pr[:], sp[:], Exp,
                                               scale=SCALE / SKV)
                      else:
                          nc.scalar.activation(
                              pr[:, 0:SCF * 32], sp[:, 0:SCF * 32],
                              Exp, scale=SCALE / SKV)
                          nc.scalar.activation(
                              pr[0:wlen, SCF * 32:COLS],
                              sp[0:wlen, SCF * 32:COLS],
                              Exp, scale=SCALE / SKV)
                      return pr

                  def emit_den_av(b, h, pr):
                      vt = VTS[b]
                      o = OFF[b]
                      if h == 0:
                          aps_of[b] = av_ps.tile([128, 256], f32, tag="ap",
                                                 name="ap")
                      ap = aps_of[b]
                      dps = ap[:, 128 + h * 32:128 + (h + 1) * 32]
                      for sc in range(SCF):
                          nc.tensor.matmul(
                              dps, ones_mat[:],
                              pr[:, sc * 32:(sc + 1) * 32],
                              start=(sc == 0), stop=False)
                      nc.tensor.matmul(
                          dps, ones_mat[o:o + 32, :],
                          pr[o:o + 32, SCF * 32:COLS],
                          start=False, stop=True)

                      aps = ap[:, h * 32:(h + 1) * 32]
                      for sc in range(SCF):
                          nc.tensor.matmul(
                              aps,
                              vt[:, h * SCF * HD + sc * HD:
                                 h * SCF * HD + (sc + 1) * HD],
                              pr[:, sc * 32:(sc + 1) * 32],
                              start=(sc == 0), stop=False)
                      vn, vo = vnb_new(b)
                      nc.tensor.matmul(
                          aps, vn[vo:vo + 32, h * 128:(h + 1) * 128],
                          pr[o:o + 32, SCF * 32:COLS],
                          start=False, stop=True)

                  def emit_quant_b(b0):
                      # normalize + hi/lo quantize batch b0 (DVE)
                      for h in range(HL):
                          rr = rb_pool.tile([128, 32], f32, tag="rr",
                                            name="rr")
                          nc.vector.reciprocal(
                              rr[:],
                              ATH3[:, b0 * 256 + 128 + h * 32:
                                   b0 * 256 + 128 + (h + 1) * 32])
                          pt = pt_pool.tile([128, 32], bf16, tag="pt",
                                            name="pt")
                          nc.vector.tensor_mul(
                              pt[:],
                              ATH3[:, b0 * 256 + h * 32:
                                   b0 * 256 + (h + 1) * 32],
                              rr[:])
                          g, s = h // 2, h % 2
                          lo, hi = b0 * 32, (b0 + 1) * 32
                          ah = AH2[g][:].rearrange(
                              "p (a b) -> p a b", a=2)[:, s, lo:hi]
                          al = AL2[g][:].rearrange(
                              "p (a b) -> p a b", a=2)[:, s, lo:hi]
                          nc.vector.tensor_scalar_mul(ah, pt[:], SA / SKV)
                          nc.vector.scalar_tensor_tensor(
                              al, pt[:], SA / SKV, ah,
                              op0=mybir.AluOpType.mult,
                              op1=mybir.AluOpType.subtract)

                  # ot tiles span a (D-half, token-half); filled by wo
                  # passes, DMAed once per half (512B-contiguous rows)
                  ot_of = {}

                  def emit_wo_pass(lo, width, final):
                      # project tokens [lo, lo+width) through wo
                      ht = lo // 256             # token-half index
                      tl = lo % 256              # token offset inside ot
                      WO3 = [(WOH, AH2), (WOL, AH2), (WOH, AL2)]
                      for half in range(2):
                          if tl == 0:
                              ot_of[half] = out_pool.tile(
                                  [128, 16 * 256], bf16, tag=f"ot{half}",
                                  name=f"ot{half}")
                          ot = ot_of[half]
                          otr = ot[:].rearrange("p (m t) -> p m t", m=16)
                          for jj in range(4):
                              po = po_ps.tile([128, 4 * width], f32,
                                              tag="po", name="po")
                              for mcj in range(4):
                                  mc = half * 16 + jj * 4 + mcj
                                  for pi, (wt, at) in enumerate(WO3):
                                      for g in range(2):
                                          nc.tensor.matmul(
                                              po[:, mcj * width:
                                                 (mcj + 1) * width],
                                              wt[:].rearrange(
                                                  "p (a b) -> p a b",
                                                  a=HL)[
                                                  :, 2 * g:2 * g + 2,
                                                  mc * 128:(mc + 1) * 128],
                                              at[g][:].rearrange(
                                                  "p (a b) -> p a b", a=2)[
                                                  :, :, lo:lo + width],
                                              start=(pi == 0 and g == 0),
                                              stop=(pi == 2 and g == 1),
                                              perf_mode=DR)
                              dst = otr[:, jj * 4:(jj + 1) * 4,
                                        tl:tl + width]
                              if final or jj % 2 == 0:
                                  nc.vector.tensor_scalar_mul(
                                      dst, po[:].rearrange(
                                          "p (a b) -> p a b", a=4),
                                      1.0 / (SW * SA))
                              else:
                                  nc.scalar.mul(
                                      dst, po[:].rearrange(
                                          "p (a b) -> p a b", a=4),
                                      1.0 / (SW * SA))
                          if tl + width == 256:
                              dst = outp[half * 2048:(half + 1) * 2048,
                                         ht * 256:(ht + 1) * 256]
                              if final:
                                  # per-jj pieces on rotating queues so
                                  # each piece ships as soon as its own
                                  # evictions land
                                  engs = [nc.sync, nc.scalar, nc.gpsimd]
                                  for jj in range(4):
                                      engs[(half * 4 + jj) % 3].dma_start(
                                          dst[jj * 512:(jj + 1) * 512, :]
                                          .rearrange("(m p) t -> p m t",
                                                     p=128),
                                          otr[:, jj * 4:(jj + 1) * 4, :])
                              else:
                                  nc.sync.dma_start(
                                      dst.rearrange("(m p) t -> p m t",
                                                    p=128),
                                      otr)

                  # software-pipelined (b, h) loop: scores/exp of pair i+1
                  # are emitted before den/av of pair i to keep PE busy
                  # while exp runs on Act.
                  pending = []   # [(b, h, pr)]

                  def flush_one():
                      b0, h0, pr0 = pending.pop(0)
                      emit_den_av(b0, h0, pr0)
                      if h0 == HL - 1:
                          nc.vector.tensor_copy(
                              ATH3[:, b0 * 256:(b0 + 1) * 256],
                              aps_of[b0][:])
                          emit_quant_b(b0)
                          if b0 < HL:
                              # SBUF->SBUF re-base of VNB4[b0] rows 96:128
                              # (needed by batch 4*b0+3's new-token A.V)
                              t, o = ((VNB3A, 32 * b0) if b0 < 3
                                      else (VNB3B, 0))
                              nc.vector.tensor_copy(
                                  t[o:o + 32, :], VNB4[b0][96:128, :])
                          if b0 < 2:
                              emit_wo_pass(32 * b0, 32, final=False)
                          elif b0 % 2 == 1:
                              emit_wo_pass(32 * (b0 - 1), 64,
                                           final=(b0 == 15))

                  for b in range(B):
                      for h in range(HL):
                          pr = emit_scores_exp(b, h)
                          if len(pending) >= 2:
                              flush_one()
                          pending.append((b, h, pr))
                  while pending:
                      flush_one()

    nc.compile()
    return nc


def _host_prep(x, wq, wk, wv, wo, k_cache, v_cache, n_cached):
    import ml_dtypes
    E4 = ml_dtypes.float8_e4m3
    E3 = ml_dtypes.float8_e3m4

    x = np.ascontiguousarray(np.asarray(x, dtype=np.float32))
    wq = np.asarray(wq, dtype=np.float32)
    wk = np.asarray(wk, dtype=np.float32)
    wv = np.asarray(wv, dtype=np.float32)
    wo = np.asarray(wo, dtype=np.float32)
    k_cache = np.asarray(k_cache, dtype=np.float32)
    v_cache = np.asarray(v_cache, dtype=np.float32)

    SCF = n_cached // 128
    perm = np.concatenate([np.arange(0, HD, 2), np.arange(1, HD, 2)])

    xt = np.ascontiguousarray(x.reshape(NTOK, D).T)  # [D, NTOK]
    xt_hi = xt.astype(E4)
    xt_lo = np.ascontiguousarray(xt - xt_hi.astype(np.float32)).astype(E4)

    def hilo(w):
        ws = w * np.float32(SW)
        hi = ws.astype(E4)
        lo = np.ascontiguousarray(ws - hi.astype(np.float32)).astype(E4)
        return np.ascontiguousarray(hi), lo

    # rope tables in deinterleaved layout, carrying the 1/SW fold
    theta = (np.float32(10000.0) **
             (np.float32(-2.0) * np.arange(0, HD, 2, dtype=np.float32)
              / np.float32(HD)))                      # [64]
    freqs = np.arange(T, dtype=np.float32)[:, None] * theta[None, :]  # [T,64]
    cos_t = np.cos(freqs).astype(np.float32).T        # [64, T]
    sin_t = np.sin(freqs).astype(np.float32).T
    cos_rep = np.tile(cos_t, (1, B)) / np.float32(SW)
    sin_rep = np.tile(sin_t, (1, B)) / np.float32(SW)
    cosd = np.ascontiguousarray(np.concatenate([cos_rep, cos_rep], axis=0))
    sind = np.ascontiguousarray(np.concatenate([-sin_rep, sin_rep], axis=0))
    # K tables carry xSKV so cached and new scores share one exp() scale
    coskd = np.ascontiguousarray(cosd * np.float32(SKV))
    sinkd = np.ascontiguousarray(sind * np.float32(SKV))

    in_maps = []
    for c in range(NC):
        hs = np.arange(c * HL, (c + 1) * HL)
        cols = (hs[:, None] * HD + perm[None, :]).reshape(-1)   # permuted q/k
        colsv = (hs[:, None] * HD + np.arange(HD)[None, :]).reshape(-1)
        wq_hi, wq_lo = hilo(wq[:, cols])
        wk_hi, wk_lo = hilo(wk[:, cols])
        wv_hi, wv_lo = hilo(wv[:, colsv])
        wo_hi, wo_lo = hilo(wo[colsv, :])
        # k cache: [b, h, hd(perm), s], x SKV
        kc_c = np.ascontiguousarray(
            k_cache[:, :n_cached][:, :, hs][:, :, :, perm]
            .transpose(0, 2, 3, 1) * np.float32(SKV)).astype(E3)
        # v cache: [b, h, sp, sc, hd] -> flat [b, h, 128, SCF*HD], x SKV
        vc_c = np.ascontiguousarray(
            v_cache[:, :n_cached][:, :, hs]
            .reshape(B, SCF, 128, HL, HD)
            .transpose(0, 3, 2, 1, 4)
            .reshape(B, HL, 128, SCF * HD) * np.float32(SKV)).astype(E3)
        in_maps.append({
            "xh": xt_hi, "xl": xt_lo,
            "wqh": wq_hi, "wql": wq_lo, "wkh": wk_hi, "wkl": wk_lo,
            "wvh": wv_hi, "wvl": wv_lo,
            "woh": wo_hi, "wol": wo_lo,
            "ktc": kc_c, "vcc": vc_c, "cosd": cosd, "sind": sind,
            "coskd": coskd, "sinkd": sinkd,
        })
    return in_maps


def kernel(x, wq, wk, wv, wo, k_cache, v_cache, start_pos):
    from concourse import bass_utils

    n_cached = int(start_pos)
    assert n_cached % 128 == 0, "kernel assumes start_pos multiple of 128"

    if _STATE.get("n_cached") != n_cached:
        _STATE["nc"] = _build(n_cached)
        _STATE["n_cached"] = n_cached
    ncb = _STATE["nc"]

    in_maps = _host_prep(x, wq, wk, wv, wo, k_cache, v_cache, n_cached)
    res = None
    for attempt in range(4):
        try:
            res = bass_utils.run_bass_kernel_spmd(ncb, in_maps,
                                                  core_ids=list(range(NC)))
            break
        except Exception:
            if attempt == 3:
                raise
            import time as _time
            _time.sleep(20 * (attempt + 1))
    out = np.zeros((D, NTOK), dtype=np.float32)
    for c in range(NC):
        out += res.results[c]["outp"].astype(np.float32)
    return np.ascontiguousarray(out.T).reshape(B, T, D)
